# revision 15
# baseline (speedup 1.0000x reference)
"""MoE-routed CNN expert kernel for 8 Trainium2 NeuronCores.

Problem (hardcoded shapes):
  x:                  [B=32, C=128, H=56, W=56] fp32
  gate:               [B=32, KSEL=2] int  (expert ids in [0, 8))
  conv_special_param: [E=8, KN=128, C=128, 3, 3] fp32
  conv_channel_param: [E=8, C=128, C+KN=256, 1, 1] fp32
  out:                [B=32, KSEL=2, C=128, H=56, W=56] fp32

Reference per (b, s):  e = gate[b, s]
  nf  = conv3x3(x[b], conv_special_param[e])          # [KN, H, W]
  out = conv_channel_param[e] @ concat([x[b], nf])    # 1x1 conv, [C, H, W]

Algebraic fusion done on HOST (exact, fp64 accumulate):
  out = conv3x3(x[b]; W'[e]) + Wx[e] @ x[b]
  W'[e][o,c,dy,dx] = sum_kn Wch[e][o, C+kn] * Wsp[e][kn,c,dy,dx]
  Wx[e][o,c]       = Wch[e][o, c]
  and Wx folds into the (1,1) center tap of W' (identity alignment on the
  padded input), so the device does only a 9-tap routed conv via
  9 x 7 fp32r matmuls per (b, s) pair accumulated in PSUM.

Sharding: data-parallel over B; core m handles samples 4m..4m+4 (8 pairs).
Routing (gate) is resolved on host by gathering per-pair weight tables.
"""

import numpy as np

B, C, H, W = 32, 128, 56, 56
KSEL = 2
E = 8
KN = 128
KS = 3
N_CORES = 8
SAMPLES_PER_CORE = B // N_CORES          # 4
PAIRS_PER_CORE = SAMPLES_PER_CORE * KSEL  # 8
HP, WP = H + 2, W + 2                    # 58, 58 padded
HW = H * W                               # 3136
N_GROUPS = KS * KS                       # 9 taps (residual folded into center)
H_TILE = 8                               # rows per PSUM tile
N_TILES = H // H_TILE                    # 7
FREE = H_TILE * W                        # 448 (<= 512 fp32 PSUM bank)

_COMPILED = None  # cached (nc, core_ids) - program is input-independent


def _build_program(loop_n=None):
    """loop_n: if set, wrap the whole body in a runtime For_i loop repeating
    it loop_n times (used only for wall-clock hardware calibration)."""
    import contextlib

    import concourse.tile as tile
    from concourse import bacc, mybir

    nc = bacc.Bacc(
        "TRN2", target_bir_lowering=False, debug=False, num_devices=N_CORES
    )
    f32 = mybir.dt.float32
    f32r = mybir.dt.float32r

    xp_d = nc.dram_tensor(
        "xp", [SAMPLES_PER_CORE, C, HP, WP], f32r, kind="ExternalInput"
    )
    wt_d = nc.dram_tensor(
        "wt", [PAIRS_PER_CORE, C, N_GROUPS * C], f32r, kind="ExternalInput"
    )
    out_d = nc.dram_tensor(
        "out", [PAIRS_PER_CORE, C, HW], f32, kind="ExternalOutput"
    )
    xp_ap = xp_d.ap()
    wt_ap = wt_d.ap()
    out_ap = out_d.ap()

    with tile.TileContext(nc) as tc:
        loop_cm = (
            tc.For_i(0, loop_n, 1, hint_engines=(mybir.EngineType.PE,))
            if loop_n is not None
            else contextlib.nullcontext()
        )
        with (
            loop_cm,
            tc.tile_pool(name="xpool", bufs=2) as xpool,
            tc.tile_pool(name="wpool", bufs=3) as wpool,
            tc.tile_pool(name="opool", bufs=3) as opool,
            tc.tile_pool(name="psum", bufs=8, space="PSUM") as psum_pool,
        ):
            # row-chunked x loads: h-tile t reads rows [8t, 8t+10), so each
            # chunk below fully covers a run of h-tiles, letting the first
            # matmuls start before the whole sample has landed. The first
            # sample uses a small leading chunk to minimize the startup gap.
            for lb in range(SAMPLES_PER_CORE):
                # critical path on sample 0: first pair's weights land first,
                # then x in row chunks (h-tile t reads rows [8t, 8t+10)) so
                # the first matmuls start as soon as rows 0..10 arrive.
                x_chunks = (
                    [(0, 10), (10, 26), (26, 42), (42, HP)]
                    if lb == 0
                    else [(0, HP)]
                )
                xt = xpool.tile([C, HP, WP], f32r)
                w0 = wpool.tile([C, N_GROUPS * C], f32r, name="wtile0")
                if lb == 0:
                    # startup critical path: tap-0 weights + x rows [0,10)
                    # are all the very first matmul needs
                    nc.sync.dma_start(out=w0[:, :C], in_=wt_ap[lb * KSEL][:, :C])
                    nc.sync.dma_start(
                        out=xt[:, 0:10, :], in_=xp_ap[lb, :, 0:10, :]
                    )
                    nc.sync.dma_start(out=w0[:, C:], in_=wt_ap[lb * KSEL][:, C:])
                    x_chunks = x_chunks[1:]
                else:
                    nc.sync.dma_start(out=w0[:], in_=wt_ap[lb * KSEL])
                for r0, r1 in x_chunks:
                    nc.sync.dma_start(
                        out=xt[:, r0:r1, :], in_=xp_ap[lb, :, r0:r1, :]
                    )
                w1 = wpool.tile([C, N_GROUPS * C], f32r, name="wtile1")
                nc.sync.dma_start(out=w1[:], in_=wt_ap[lb * KSEL + 1])

                for s, wtile in enumerate((w0, w1)):
                    p = lb * KSEL + s
                    ot = opool.tile([C, HW], f32)
                    for t in range(N_TILES):
                        pst = psum_pool.tile([C, FREE], f32, name=f"ps{t}", tag="ps")
                        for g in range(N_GROUPS):
                            dy, dx = g // KS, g % KS
                            rhs = xt[
                                :, t * H_TILE + dy : t * H_TILE + dy + H_TILE,
                                dx : dx + W,
                            ]
                            nc.tensor.matmul(
                                pst[:],
                                wtile[:, g * C : (g + 1) * C],
                                rhs,
                                start=(g == 0),
                                stop=(g == N_GROUPS - 1),
                            )
                        # DVE copies are ~3x faster than ACT for fp32 PSUM
                        # reads; DVE has no other work so it takes all of them.
                        # out-DMAs issue from the ACT HWDGE queue to keep the
                        # SP queue free for input loads. The very last tile is
                        # drained in halves to shorten the kernel-exit tail.
                        last = p == PAIRS_PER_CORE - 1 and t == N_TILES - 1
                        for f0, f1 in ([(0, FREE // 2), (FREE // 2, FREE)] if last else [(0, FREE)]):
                            dst = ot[:, t * FREE + f0 : t * FREE + f1]
                            nc.vector.tensor_copy(dst, pst[:, f0:f1])
                            nc.scalar.dma_start(
                                out=out_ap[p][:, t * FREE + f0 : t * FREE + f1],
                                in_=dst,
                            )

    nc.compile()
    return nc


def _get_compiled():
    global _COMPILED
    if _COMPILED is None:
        _COMPILED = _build_program()
    return _COMPILED


def _fused_weight_tables(conv_special_param, conv_channel_param):
    """Per-expert lhsT tables L[e][c, g*C + o], residual folded into center tap."""
    wsp = np.asarray(conv_special_param, dtype=np.float64)  # [E, KN, C, 3, 3]
    wch = np.asarray(conv_channel_param, dtype=np.float64)[..., 0, 0]  # [E, C, C+KN]
    wch_x = wch[:, :, :C]                  # [E, o, c]
    wch_nf = wch[:, :, C:]                 # [E, o, kn]
    # W'[e][o, c, dy, dx] = sum_kn wch_nf[e][o, kn] * wsp[e][kn, c, dy, dx]
    wf = np.einsum("eok,ekcyx->eocyx", wch_nf, wsp)  # [E, o, c, 3, 3]
    wf[:, :, :, 1, 1] += wch_x                        # fold residual into center
    # lhsT layout: [e, c, g, o] with g = dy*3 + dx
    L = wf.transpose(0, 2, 3, 4, 1).reshape(E, C, N_GROUPS, C)
    return np.ascontiguousarray(L.reshape(E, C, N_GROUPS * C), dtype=np.float32)


def kernel(x, gate, conv_special_param, conv_channel_param):
    from concourse.bass_utils import run_bass_kernel_spmd

    x = np.asarray(x, dtype=np.float32)
    gate = np.asarray(gate)
    assert x.shape == (B, C, H, W)
    assert gate.shape == (B, KSEL)

    L = _fused_weight_tables(conv_special_param, conv_channel_param)

    xp = np.zeros((B, C, HP, WP), dtype=np.float32)
    xp[:, :, 1 : 1 + H, 1 : 1 + W] = x

    gate_i = gate.astype(np.int64)
    in_maps = []
    for m in range(N_CORES):
        b0 = m * SAMPLES_PER_CORE
        # wt[p] for p = local_b * KSEL + s
        eids = gate_i[b0 : b0 + SAMPLES_PER_CORE].reshape(-1)  # [8]
        in_maps.append(
            {
                "xp": np.ascontiguousarray(xp[b0 : b0 + SAMPLES_PER_CORE]),
                "wt": np.ascontiguousarray(L[eids]),
            }
        )

    nc = _get_compiled()
    res = run_bass_kernel_spmd(nc, in_maps, list(range(N_CORES)))

    out = np.empty((B, KSEL, C, H, W), dtype=np.float32)
    for m in range(N_CORES):
        b0 = m * SAMPLES_PER_CORE
        o = res.results[m]["out"].reshape(SAMPLES_PER_CORE, KSEL, C, H, W)
        out[b0 : b0 + SAMPLES_PER_CORE] = o
    return out


# revision 16
# speedup vs baseline: 1.0020x; 1.0020x over previous
"""MoE-routed CNN expert kernel for 8 Trainium2 NeuronCores.

Problem (hardcoded shapes):
  x:                  [B=32, C=128, H=56, W=56] fp32
  gate:               [B=32, KSEL=2] int  (expert ids in [0, 8))
  conv_special_param: [E=8, KN=128, C=128, 3, 3] fp32
  conv_channel_param: [E=8, C=128, C+KN=256, 1, 1] fp32
  out:                [B=32, KSEL=2, C=128, H=56, W=56] fp32

Reference per (b, s):  e = gate[b, s]
  nf  = conv3x3(x[b], conv_special_param[e])          # [KN, H, W]
  out = conv_channel_param[e] @ concat([x[b], nf])    # 1x1 conv, [C, H, W]

Algebraic fusion done on HOST (exact, fp64 accumulate):
  out = conv3x3(x[b]; W'[e]) + Wx[e] @ x[b]
  W'[e][o,c,dy,dx] = sum_kn Wch[e][o, C+kn] * Wsp[e][kn,c,dy,dx]
  Wx[e][o,c]       = Wch[e][o, c]
  and Wx folds into the (1,1) center tap of W' (identity alignment on the
  padded input), so the device does only a 9-tap routed conv via
  9 x 7 fp32r matmuls per (b, s) pair accumulated in PSUM.

Sharding: data-parallel over B; core m handles samples 4m..4m+4 (8 pairs).
Routing (gate) is resolved on host by gathering per-pair weight tables.
"""

import numpy as np

B, C, H, W = 32, 128, 56, 56
KSEL = 2
E = 8
KN = 128
KS = 3
N_CORES = 8
SAMPLES_PER_CORE = B // N_CORES          # 4
PAIRS_PER_CORE = SAMPLES_PER_CORE * KSEL  # 8
HP, WP = H + 2, W + 2                    # 58, 58 padded
HW = H * W                               # 3136
N_GROUPS = KS * KS                       # 9 taps (residual folded into center)
H_TILE = 8                               # rows per PSUM tile
N_TILES = H // H_TILE                    # 7
FREE = H_TILE * W                        # 448 (<= 512 fp32 PSUM bank)

_COMPILED = None  # cached (nc, core_ids) - program is input-independent


def _build_program(loop_n=None):
    """loop_n: if set, wrap the whole body in a runtime For_i loop repeating
    it loop_n times (used only for wall-clock hardware calibration)."""
    import contextlib

    import concourse.tile as tile
    from concourse import bacc, mybir

    nc = bacc.Bacc(
        "TRN2", target_bir_lowering=False, debug=False, num_devices=N_CORES
    )
    f32 = mybir.dt.float32
    f32r = mybir.dt.float32r

    xp_d = nc.dram_tensor(
        "xp", [SAMPLES_PER_CORE, C, HP, WP], f32r, kind="ExternalInput"
    )
    wt_d = nc.dram_tensor(
        "wt", [PAIRS_PER_CORE, C, N_GROUPS * C], f32r, kind="ExternalInput"
    )
    out_d = nc.dram_tensor(
        "out", [PAIRS_PER_CORE, C, HW], f32, kind="ExternalOutput"
    )
    xp_ap = xp_d.ap()
    wt_ap = wt_d.ap()
    out_ap = out_d.ap()

    with tile.TileContext(nc) as tc:
        loop_cm = (
            tc.For_i(0, loop_n, 1, hint_engines=(mybir.EngineType.PE,))
            if loop_n is not None
            else contextlib.nullcontext()
        )
        with (
            loop_cm,
            tc.tile_pool(name="xpool", bufs=2) as xpool,
            tc.tile_pool(name="wpool", bufs=3) as wpool,
            tc.tile_pool(name="opool", bufs=3) as opool,
            tc.tile_pool(name="psum", bufs=8, space="PSUM") as psum_pool,
        ):
            # row-chunked x loads: h-tile t reads rows [8t, 8t+10), so each
            # chunk below fully covers a run of h-tiles, letting the first
            # matmuls start before the whole sample has landed. The first
            # sample uses a small leading chunk to minimize the startup gap.
            for lb in range(SAMPLES_PER_CORE):
                # critical path on sample 0: first pair's weights land first,
                # then x in row chunks (h-tile t reads rows [8t, 8t+10)) so
                # the first matmuls start as soon as rows 0..10 arrive.
                x_chunks = (
                    [(0, 10), (10, 26), (26, 42), (42, HP)]
                    if lb == 0
                    else [(0, HP)]
                )
                xt = xpool.tile([C, HP, WP], f32r)
                w0 = wpool.tile([C, N_GROUPS * C], f32r, name="wtile0")
                if lb == 0:
                    # startup critical path: tap-0 weights + x rows [0,10)
                    # are all the very first matmul needs
                    nc.sync.dma_start(out=w0[:, :C], in_=wt_ap[lb * KSEL][:, :C])
                    nc.sync.dma_start(
                        out=xt[:, 0:10, :], in_=xp_ap[lb, :, 0:10, :]
                    )
                    nc.sync.dma_start(out=w0[:, C:], in_=wt_ap[lb * KSEL][:, C:])
                    x_chunks = x_chunks[1:]
                else:
                    nc.sync.dma_start(out=w0[:], in_=wt_ap[lb * KSEL])
                for r0, r1 in x_chunks:
                    nc.sync.dma_start(
                        out=xt[:, r0:r1, :], in_=xp_ap[lb, :, r0:r1, :]
                    )
                w1 = wpool.tile([C, N_GROUPS * C], f32r, name="wtile1")
                nc.sync.dma_start(out=w1[:], in_=wt_ap[lb * KSEL + 1])

                for s, wtile in enumerate((w0, w1)):
                    p = lb * KSEL + s
                    ot = opool.tile([C, HW], f32)
                    for t in range(N_TILES):
                        pst = psum_pool.tile([C, FREE], f32, name=f"ps{t}", tag="ps")
                        for g in range(N_GROUPS):
                            dy, dx = g // KS, g % KS
                            rhs = xt[
                                :, t * H_TILE + dy : t * H_TILE + dy + H_TILE,
                                dx : dx + W,
                            ]
                            nc.tensor.matmul(
                                pst[:],
                                wtile[:, g * C : (g + 1) * C],
                                rhs,
                                start=(g == 0),
                                stop=(g == N_GROUPS - 1),
                            )
                        # DVE copies are ~3x faster than ACT for fp32 PSUM
                        # reads; DVE has no other work so it takes all of them.
                        dst = ot[:, t * FREE : (t + 1) * FREE]
                        nc.vector.tensor_copy(dst, pst[:])
                        # out-DMAs issue from the ACT HWDGE queue to keep the
                        # SP queue free for input loads.
                        nc.scalar.dma_start(
                            out=out_ap[p][:, t * FREE : (t + 1) * FREE], in_=dst
                        )

    nc.compile()
    return nc


def _get_compiled():
    global _COMPILED
    if _COMPILED is None:
        _COMPILED = _build_program()
    return _COMPILED


def _fused_weight_tables(conv_special_param, conv_channel_param):
    """Per-expert lhsT tables L[e][c, g*C + o], residual folded into center tap."""
    wsp = np.asarray(conv_special_param, dtype=np.float64)  # [E, KN, C, 3, 3]
    wch = np.asarray(conv_channel_param, dtype=np.float64)[..., 0, 0]  # [E, C, C+KN]
    wch_x = wch[:, :, :C]                  # [E, o, c]
    wch_nf = wch[:, :, C:]                 # [E, o, kn]
    # W'[e][o, c, dy, dx] = sum_kn wch_nf[e][o, kn] * wsp[e][kn, c, dy, dx]
    wf = np.einsum("eok,ekcyx->eocyx", wch_nf, wsp)  # [E, o, c, 3, 3]
    wf[:, :, :, 1, 1] += wch_x                        # fold residual into center
    # lhsT layout: [e, c, g, o] with g = dy*3 + dx
    L = wf.transpose(0, 2, 3, 4, 1).reshape(E, C, N_GROUPS, C)
    return np.ascontiguousarray(L.reshape(E, C, N_GROUPS * C), dtype=np.float32)


def kernel(x, gate, conv_special_param, conv_channel_param):
    from concourse.bass_utils import run_bass_kernel_spmd

    x = np.asarray(x, dtype=np.float32)
    gate = np.asarray(gate)
    assert x.shape == (B, C, H, W)
    assert gate.shape == (B, KSEL)

    L = _fused_weight_tables(conv_special_param, conv_channel_param)

    xp = np.zeros((B, C, HP, WP), dtype=np.float32)
    xp[:, :, 1 : 1 + H, 1 : 1 + W] = x

    gate_i = gate.astype(np.int64)
    in_maps = []
    for m in range(N_CORES):
        b0 = m * SAMPLES_PER_CORE
        # wt[p] for p = local_b * KSEL + s
        eids = gate_i[b0 : b0 + SAMPLES_PER_CORE].reshape(-1)  # [8]
        in_maps.append(
            {
                "xp": np.ascontiguousarray(xp[b0 : b0 + SAMPLES_PER_CORE]),
                "wt": np.ascontiguousarray(L[eids]),
            }
        )

    nc = _get_compiled()
    res = run_bass_kernel_spmd(nc, in_maps, list(range(N_CORES)))

    out = np.empty((B, KSEL, C, H, W), dtype=np.float32)
    for m in range(N_CORES):
        b0 = m * SAMPLES_PER_CORE
        o = res.results[m]["out"].reshape(SAMPLES_PER_CORE, KSEL, C, H, W)
        out[b0 : b0 + SAMPLES_PER_CORE] = o
    return out


# revision 22
# speedup vs baseline: 1.0726x; 1.0705x over previous
"""MoE-routed CNN expert kernel for 8 Trainium2 NeuronCores.

Problem (hardcoded shapes):
  x:                  [B=32, C=128, H=56, W=56] fp32
  gate:               [B=32, KSEL=2] int  (expert ids in [0, 8))
  conv_special_param: [E=8, KN=128, C=128, 3, 3] fp32
  conv_channel_param: [E=8, C=128, C+KN=256, 1, 1] fp32
  out:                [B=32, KSEL=2, C=128, H=56, W=56] fp32

Reference per (b, s):  e = gate[b, s]
  nf  = conv3x3(x[b], conv_special_param[e])          # [KN, H, W]
  out = conv_channel_param[e] @ concat([x[b], nf])    # 1x1 conv, [C, H, W]

Algebraic fusion done on HOST (exact, fp64 accumulate):
  out = conv3x3(x[b]; W'[e]) + Wx[e] @ x[b]
  W'[e][o,c,dy,dx] = sum_kn Wch[e][o, C+kn] * Wsp[e][kn,c,dy,dx]
  Wx[e][o,c]       = Wch[e][o, c]
  and Wx folds into the (1,1) center tap of W' (identity alignment on the
  padded input), so the device does only a 9-tap routed conv via
  9 x 7 fp32r matmuls per (b, s) pair accumulated in PSUM.

Sharding: data-parallel over B; core m handles samples 4m..4m+4 (8 pairs).
Routing (gate) is resolved on host by gathering per-pair weight tables.
"""

import numpy as np

B, C, H, W = 32, 128, 56, 56
KSEL = 2
E = 8
KN = 128
KS = 3
N_CORES = 8
SAMPLES_PER_CORE = B // N_CORES          # 4
PAIRS_PER_CORE = SAMPLES_PER_CORE * KSEL  # 8
HP, WP = H + 2, W + 2                    # 58, 58 padded
HW = H * W                               # 3136
N_GROUPS = KS * KS                       # 9 taps (residual folded into center)
H_TILE = 8                               # rows per PSUM tile
N_TILES = H // H_TILE                    # 7
FREE = H_TILE * W                        # 448 (<= 512 fp32 PSUM bank)
N_WARMUP_MM = 8                          # sacrificial matmuls to warm PE clock

_COMPILED = None  # cached (nc, core_ids) - program is input-independent


def _build_program(loop_n=None):
    """loop_n: if set, wrap the whole body in a runtime For_i loop repeating
    it loop_n times (used only for wall-clock hardware calibration)."""
    import contextlib

    import concourse.tile as tile
    from concourse import bacc, mybir

    nc = bacc.Bacc(
        "TRN2", target_bir_lowering=False, debug=False, num_devices=N_CORES
    )
    f32 = mybir.dt.float32
    f32r = mybir.dt.float32r

    xp_d = nc.dram_tensor(
        "xp", [SAMPLES_PER_CORE, C, HP, WP], f32r, kind="ExternalInput"
    )
    wt_d = nc.dram_tensor(
        "wt", [PAIRS_PER_CORE, C, N_GROUPS * C], f32r, kind="ExternalInput"
    )
    out_d = nc.dram_tensor(
        "out", [PAIRS_PER_CORE, C, HW], f32, kind="ExternalOutput"
    )
    xp_ap = xp_d.ap()
    wt_ap = wt_d.ap()
    out_ap = out_d.ap()

    with tile.TileContext(nc) as tc:
        loop_cm = (
            tc.For_i(0, loop_n, 1, hint_engines=(mybir.EngineType.PE,))
            if loop_n is not None
            else contextlib.nullcontext()
        )
        with (
            loop_cm,
            tc.tile_pool(name="xpool", bufs=2) as xpool,
            tc.tile_pool(name="wpool", bufs=3) as wpool,
            tc.tile_pool(name="opool", bufs=3) as opool,
            tc.tile_pool(name="psum", bufs=8, space="PSUM") as psum_pool,
        ):
            # row-chunked x loads: h-tile t reads rows [8t, 8t+10), so each
            # chunk below fully covers a run of h-tiles, letting the first
            # matmuls start before the whole sample has landed. The first
            # sample uses a small leading chunk to minimize the startup gap.
            # PE clock-gate warmup: the tensor engine starts throttled
            # (1.2 GHz) until it has been busy ~3.4us. Fill the initial
            # DMA-wait window with sacrificial matmuls on a zeroed tile so
            # the real matmuls start at full clock.
            if N_WARMUP_MM > 0:
                wz = wpool.tile([C, C + FREE], f32, name="warm_z")
                nc.vector.memset(wz[:], 0.0)
                wsrc = wpool.tile([C, C + FREE], f32r, name="warm_src")
                nc.vector.tensor_copy(wsrc[:], wz[:])
                warm_ps = psum_pool.tile([C, FREE], f32, name="warm", tag="ps")
                for _ in range(N_WARMUP_MM):
                    nc.tensor.matmul(
                        warm_ps[:],
                        wsrc[:, :C],
                        wsrc[:, C : C + FREE],
                        start=True,
                        stop=True,
                    )

            for lb in range(SAMPLES_PER_CORE):
                # critical path on sample 0: first pair's weights land first,
                # then x in row chunks (h-tile t reads rows [8t, 8t+10)) so
                # the first matmuls start as soon as rows 0..10 arrive.
                x_chunks = (
                    [(0, 10), (10, 26), (26, 42), (42, HP)]
                    if lb == 0
                    else [(0, HP)]
                )
                xt = xpool.tile([C, HP, WP], f32r)
                w0 = wpool.tile([C, N_GROUPS * C], f32r, name="wtile0")
                if lb == 0:
                    # startup critical path: tap-0 weights + x rows [0,10)
                    # are all the very first matmul needs
                    nc.sync.dma_start(out=w0[:, :C], in_=wt_ap[lb * KSEL][:, :C])
                    nc.sync.dma_start(
                        out=xt[:, 0:10, :], in_=xp_ap[lb, :, 0:10, :]
                    )
                    nc.sync.dma_start(out=w0[:, C:], in_=wt_ap[lb * KSEL][:, C:])
                    x_chunks = x_chunks[1:]
                else:
                    nc.sync.dma_start(out=w0[:], in_=wt_ap[lb * KSEL])
                for r0, r1 in x_chunks:
                    nc.sync.dma_start(
                        out=xt[:, r0:r1, :], in_=xp_ap[lb, :, r0:r1, :]
                    )
                w1 = wpool.tile([C, N_GROUPS * C], f32r, name="wtile1")
                nc.sync.dma_start(out=w1[:], in_=wt_ap[lb * KSEL + 1])

                for s, wtile in enumerate((w0, w1)):
                    p = lb * KSEL + s
                    ot = opool.tile([C, HW], f32)
                    for t in range(N_TILES):
                        pst = psum_pool.tile([C, FREE], f32, name=f"ps{t}", tag="ps")
                        for g in range(N_GROUPS):
                            dy, dx = g // KS, g % KS
                            rhs = xt[
                                :, t * H_TILE + dy : t * H_TILE + dy + H_TILE,
                                dx : dx + W,
                            ]
                            nc.tensor.matmul(
                                pst[:],
                                wtile[:, g * C : (g + 1) * C],
                                rhs,
                                start=(g == 0),
                                stop=(g == N_GROUPS - 1),
                            )
                        # DVE copies are ~3x faster than ACT for fp32 PSUM
                        # reads; DVE has no other work so it takes all of them.
                        dst = ot[:, t * FREE : (t + 1) * FREE]
                        nc.vector.tensor_copy(dst, pst[:])
                        # out-DMAs issue from the ACT HWDGE queue to keep the
                        # SP queue free for input loads.
                        nc.scalar.dma_start(
                            out=out_ap[p][:, t * FREE : (t + 1) * FREE], in_=dst
                        )

    nc.compile()
    return nc


def _get_compiled():
    global _COMPILED
    if _COMPILED is None:
        _COMPILED = _build_program()
    return _COMPILED


def _fused_weight_tables(conv_special_param, conv_channel_param):
    """Per-expert lhsT tables L[e][c, g*C + o], residual folded into center tap."""
    wsp = np.asarray(conv_special_param, dtype=np.float64)  # [E, KN, C, 3, 3]
    wch = np.asarray(conv_channel_param, dtype=np.float64)[..., 0, 0]  # [E, C, C+KN]
    wch_x = wch[:, :, :C]                  # [E, o, c]
    wch_nf = wch[:, :, C:]                 # [E, o, kn]
    # W'[e][o, c, dy, dx] = sum_kn wch_nf[e][o, kn] * wsp[e][kn, c, dy, dx]
    wf = np.einsum("eok,ekcyx->eocyx", wch_nf, wsp)  # [E, o, c, 3, 3]
    wf[:, :, :, 1, 1] += wch_x                        # fold residual into center
    # lhsT layout: [e, c, g, o] with g = dy*3 + dx
    L = wf.transpose(0, 2, 3, 4, 1).reshape(E, C, N_GROUPS, C)
    return np.ascontiguousarray(L.reshape(E, C, N_GROUPS * C), dtype=np.float32)


def kernel(x, gate, conv_special_param, conv_channel_param):
    from concourse.bass_utils import run_bass_kernel_spmd

    x = np.asarray(x, dtype=np.float32)
    gate = np.asarray(gate)
    assert x.shape == (B, C, H, W)
    assert gate.shape == (B, KSEL)

    L = _fused_weight_tables(conv_special_param, conv_channel_param)

    xp = np.zeros((B, C, HP, WP), dtype=np.float32)
    xp[:, :, 1 : 1 + H, 1 : 1 + W] = x

    gate_i = gate.astype(np.int64)
    in_maps = []
    for m in range(N_CORES):
        b0 = m * SAMPLES_PER_CORE
        # wt[p] for p = local_b * KSEL + s
        eids = gate_i[b0 : b0 + SAMPLES_PER_CORE].reshape(-1)  # [8]
        in_maps.append(
            {
                "xp": np.ascontiguousarray(xp[b0 : b0 + SAMPLES_PER_CORE]),
                "wt": np.ascontiguousarray(L[eids]),
            }
        )

    nc = _get_compiled()
    try:
        res = run_bass_kernel_spmd(nc, in_maps, list(range(N_CORES)))
    except ModuleNotFoundError:
        # BASS_TRACE was set but this client lacks the axon NTFF profile
        # hook; rerun with tracing disabled.
        import os

        prev = os.environ.get("BASS_NEVER_TRACE")
        os.environ["BASS_NEVER_TRACE"] = "1"
        try:
            res = run_bass_kernel_spmd(nc, in_maps, list(range(N_CORES)))
        finally:
            if prev is None:
                os.environ.pop("BASS_NEVER_TRACE", None)
            else:
                os.environ["BASS_NEVER_TRACE"] = prev

    out = np.empty((B, KSEL, C, H, W), dtype=np.float32)
    for m in range(N_CORES):
        b0 = m * SAMPLES_PER_CORE
        o = res.results[m]["out"].reshape(SAMPLES_PER_CORE, KSEL, C, H, W)
        out[b0 : b0 + SAMPLES_PER_CORE] = o
    return out
